# revision 1
# baseline (speedup 1.0000x reference)
"""Trainium2 Bass kernel for: y = mish(W @ sum_L(x) + L*b).

x: [32, 1024, 2048] f32, W: [1024, 1024] f32, b: [1024] f32 -> y: [32, 1024] f32.

Sharding: data-parallel over batch across 8 NeuronCores (4 batches/core);
W replicated. Per-core: stream the 32MB x-shard from HBM in 4MB tiles,
reduce over L on VectorE+ScalarE, 128x128 PE matmuls accumulating in PSUM,
Mish epilogue on ScalarE, one contiguous store.
"""

import sys

for _p in ("/opt/trn_rl_repo",):
    if _p not in sys.path:
        sys.path.append(_p)

import numpy as np

B, C, L = 32, 1024, 2048
NCORES = 8
BLOC = B // NCORES  # batches per core
P = 128             # partitions
CB = C // P         # channel blocks

_CACHE = {}


def _patch_tile_drain():
    """Split the Tile exit-drain's sem waits into 1-wait carrier nops.

    walrus (this build) rejects instructions carrying >2 sync waits; the
    stock TileContext exit drain accumulates one wait per live proc.
    """
    import concourse.mybir as mybir
    from concourse import tile as tile_mod
    from concourse.tile import TileContext

    if getattr(TileContext, "_drain_split_patched", False):
        return
    ScopedClock = tile_mod.ScopedClock

    def _drain_and_barrier(self, tick_clock, wait_clock):
        nc = self.nc
        drain_inst = nc.sync.drain()
        wait_clock.add_sem_waits(
            drain_inst.ins, ScopedClock({None: tick_clock.global_clock})
        )
        si = drain_inst.ins.sync_info
        waits = list(si.on_wait or [])
        if len(waits) > 1:
            si.on_wait = waits[:1]
            for w in waits[1:]:
                carrier = nc.sync.nop(nofuse=True, hint="drain_wait_split")
                carrier.ins.sync_info = mybir.SyncInfo(on_wait=[w], on_update=[])
        nc.all_engine_barrier()
        assert self.sems is not None
        popped = nc._tile_sem_poison_stack.pop()
        assert popped is self._sem_poison
        nc.clear_and_free_semaphores(list(self.sems.allocated().values()))
        nc.all_engine_barrier()

    TileContext._drain_and_barrier = _drain_and_barrier
    TileContext._drain_split_patched = True


def _fix_bir_waits(bir_json: bytes) -> bytes:
    """Legalize sync waits: walrus codegen rejects instructions carrying
    more than ~2 sync waits. Move excess waits onto same-engine NoOp
    carriers inserted immediately before the instruction (engine streams
    execute in block order, so semantics are preserved)."""
    import json

    d = json.loads(bir_json)
    changed = False
    for fn in d.get("functions", []):
        for blk in fn.get("blocks", []):
            new_insts = []
            for ins in blk.get("instructions", []):
                si = ins.get("sync_info")
                waits = (si or {}).get("on_wait") or []
                if len(waits) > 1:
                    changed = True
                    for k, w in enumerate(waits[:-1]):
                        new_insts.append(
                            {
                                "debug": ins.get("debug", 0),
                                "engine": ins["engine"],
                                "ins": [],
                                "name": f"{ins['name']}-wsplit{k}",
                                "opcode": "NoOp",
                                "outs": [],
                                "sync_info": {"on_update": [], "on_wait": [w]},
                                "text_hint": "wait_split",
                            }
                        )
                    si["on_wait"] = [waits[-1]]
                new_insts.append(ins)
            blk["instructions"] = new_insts
    if not changed:
        return bir_json
    return json.dumps(d).encode()


def _patch_compile():
    """Route every BIR compile through _fix_bir_waits."""
    import concourse.bass_utils as bu

    if getattr(bu, "_wait_split_patched", False):
        return
    orig = bu.compile_bir_kernel

    def wrapped(bir_json, tmpdir, neff_name="file.neff"):
        return orig(_fix_bir_waits(bytes(bir_json)), tmpdir, neff_name=neff_name)

    bu.compile_bir_kernel = wrapped
    bu._wait_split_patched = True
    import concourse.bass2jax as b2j

    b2j.compile_bir_kernel = wrapped


def _build_nc():
    import concourse.bass as bass
    import concourse.mybir as mybir
    from concourse.tile import TileContext

    _patch_tile_drain()
    _patch_compile()
    f32 = mybir.dt.float32
    AF = mybir.ActivationFunctionType
    AX = mybir.AxisListType

    bf16 = mybir.dt.bfloat16
    OP = mybir.AluOpType

    nc = bass.Bass()
    x = nc.dram_tensor("x", [BLOC, C, L], f32, kind="ExternalInput")
    wt = nc.dram_tensor("wt", [CB, P, C], bf16, kind="ExternalInput")  # W^T tiled
    lb = nc.dram_tensor("lb", [CB, P], f32, kind="ExternalInput")      # L*b
    msk = nc.dram_tensor("msk", [CB, CB * BLOC], f32, kind="ExternalInput")
    out = nc.dram_tensor("out", [P, CB, BLOC], f32, kind="ExternalOutput")

    with TileContext(nc) as tc:
        with (
            tc.tile_pool(name="const", bufs=1) as cpool,
            tc.tile_pool(name="xp", bufs=3) as xpool,
            tc.tile_pool(name="sp", bufs=3) as spool,
            tc.tile_pool(name="ps", bufs=1, space="PSUM") as pspool,
        ):
            # W^T resident in SBUF (bf16): wt_sb[p, cb, o] = W[o, cb*P + p]
            wt_sb = cpool.tile([P, CB, C], bf16, tag="wt")
            nc.sync.dma_start(out=wt_sb[:], in_=wt.rearrange("cb p o -> p cb o"))
            lb_sb = cpool.tile([CB, P], f32, tag="lb")
            nc.sync.dma_start(out=lb_sb[:], in_=lb[:])
            msk_sb = cpool.tile([CB, CB * BLOC], f32, tag="msk")
            nc.sync.dma_start(out=msk_sb[:], in_=msk[:])
            dump = cpool.tile([P, L], f32, tag="dump")  # ScalarE reduce scratch
            y_sb = cpool.tile([P, CB, BLOC], f32, tag="y")

            # One PSUM bank holds all CB output blocks: ps[p, ob, b].
            ps = pspool.tile([P, CB, BLOC], f32, tag="acc")
            # Seed ALL biases with a single K=CB matmul (one start=True for
            # the whole bank): ps[p, (ob,b)] = sum_k Lb[k*P+p] * onehot[k, ob]
            # -- runs at kernel start, off the critical tail.
            nc.tensor.matmul(
                ps[:].rearrange("p cb b -> p (cb b)"),
                lhsT=lb_sb[:],
                rhs=msk_sb[:],
                start=True,
                stop=False,
            )

            xv = x.rearrange("b (cb p) l -> cb p b l", p=P)
            for cb in range(CB):
                xt = xpool.tile([P, BLOC, L], f32, tag="xt")
                if cb == CB - 1:
                    # split the last tile per batch so the final reduces
                    # start as soon as each 1MB slice lands
                    for b in range(BLOC):
                        nc.sync.dma_start(out=xt[:, b, :], in_=xv[cb, :, b, :])
                else:
                    nc.sync.dma_start(out=xt[:], in_=xv[cb])
                s_cb = spool.tile([P, BLOC], f32, tag="s")
                for b in range(BLOC):
                    if b % 2 == 0:
                        nc.vector.reduce_sum(
                            out=s_cb[:, b : b + 1], in_=xt[:, b, :], axis=AX.X
                        )
                    else:
                        nc.scalar.activation(
                            out=dump[:],
                            in_=xt[:, b, :],
                            func=AF.Identity,
                            accum_out=s_cb[:, b : b + 1],
                        )
                s16 = spool.tile([P, BLOC], bf16, tag="s16")
                nc.vector.tensor_copy(out=s16[:], in_=s_cb[:])
                for ob in range(CB):
                    nc.tensor.matmul(
                        ps[:, ob, :],
                        lhsT=wt_sb[:, cb, ob * P : (ob + 1) * P],
                        rhs=s16[:],
                        start=False,
                        stop=(cb == CB - 1),
                    )
            # Epilogue: ps = W@s + L*b; mish(y) = y * tanh(softplus(y)).
            # For z = min(y, 9): tanh(softplus(z)) = q/(q+2), q = p^2+2p,
            # p = e^z (no overflow, z <= 9; for y >= 9, tanh(softplus(y))
            # == 1 in f32 so the clamp is exact). y*q/(q+2) is computed as
            # y - 2*y/(q+2). Only Identity/Exp ACT funcs (one table set).
            pv = ps[:].rearrange("p cb b -> p (cb b)")
            ez = cpool.tile([P, CB * BLOC], f32, tag="ez")
            nc.vector.tensor_scalar_min(out=ez[:], in0=pv, scalar1=9.0)
            nc.scalar.activation(out=ez[:], in_=ez[:], func=AF.Exp)
            q = cpool.tile([P, CB * BLOC], f32, tag="q")
            nc.vector.scalar_tensor_tensor(
                out=q[:], in0=ez[:], scalar=2.0, in1=ez[:], op0=OP.add, op1=OP.mult
            )
            nc.vector.tensor_scalar_add(out=q[:], in0=q[:], scalar1=2.0)
            nc.vector.reciprocal(out=q[:], in_=q[:])  # q = 1/(p^2+2p+2)
            t = cpool.tile([P, CB * BLOC], f32, tag="t")
            nc.vector.tensor_mul(out=t[:], in0=pv, in1=q[:])  # t = y/(q+2)
            yv = y_sb[:].rearrange("p cb b -> p (cb b)")
            nc.vector.scalar_tensor_tensor(
                out=yv, in0=t[:], scalar=-2.0, in1=pv, op0=OP.mult, op1=OP.add
            )
            nc.sync.dma_start(out=out[:], in_=y_sb[:])
    return nc


def _get_nc():
    if "nc" not in _CACHE:
        _CACHE["nc"] = _build_nc()
    return _CACHE["nc"]


def _prep_in_maps(x, W, b):
    import ml_dtypes

    x = np.asarray(x, dtype=np.float32)
    W = np.asarray(W, dtype=np.float32)
    b = np.asarray(b, dtype=np.float32)
    wt = np.ascontiguousarray(W.T).reshape(CB, P, C).astype(ml_dtypes.bfloat16)
    lb = (np.float32(L) * b).reshape(CB, P)
    msk = np.zeros((CB, CB * BLOC), dtype=np.float32)
    for k in range(CB):
        msk[k, k * BLOC : (k + 1) * BLOC] = 1.0
    in_maps = []
    for i in range(NCORES):
        xs = np.ascontiguousarray(x[i * BLOC : (i + 1) * BLOC])
        in_maps.append({"x": xs, "wt": wt, "lb": lb, "msk": msk})
    return in_maps


def _gather(results):
    parts = []
    for r in results:
        o = r["out"]  # [P, CB, BLOC]
        parts.append(np.ascontiguousarray(o.transpose(2, 1, 0)).reshape(BLOC, C))
    return np.concatenate(parts, axis=0)


def _execute(x, W, b, **run_kwargs):
    from concourse.bass_utils import run_bass_kernel_spmd

    nc = _get_nc()
    in_maps = _prep_in_maps(x, W, b)
    res = run_bass_kernel_spmd(nc, in_maps, core_ids=list(range(NCORES)), **run_kwargs)
    return _gather(res.results), res


def kernel(x, W, b):
    y, _ = _execute(x, W, b)
    return y.astype(np.float32)

